# revision 1
# baseline (speedup 1.0000x reference)
# Dot-product attention with per-batch key masking (d2l masked_softmax
# style), distributed over 8 NeuronCores by batch:
#
#   out[b] = softmax(mask(Q[b] @ K[b]^T / sqrt(D), valid_lens[b])) @ V[b]
#
# Shapes: Q/K/V [32, 2048, 64] f32, valid_lens [32] i32.
#
# Strategy (per core: 4 batch "slots"; batches sorted by valid_len so
# slot s of every core shares a compile-time k-tile trip count kc[s]):
#   - Host pre-transposes Q, K to d-major [D, S] fp16 (layout prep), so
#     mm1 runs with the d=64 contraction on partitions 0-63: scoresT
#     [k-tile 128, q] = KT_tile.T @ QT. fp16 operands stream at the PE's
#     full 1 row/cycle (f32r measured 2 cycles/row on this HW); Q/K
#     quantization to fp16 adds ~3e-5 score error. No PE row-tiling
#     pair-packing: HW microbenches show matmuls on disjoint row groups
#     do NOT overlap, so Q is loaded once (not duplicated).
#   - attnT = Exp(scoresT/8 + mask_bias) on the ACT engine, fp16 out;
#     mask_bias is 0 / -1e6 per key so masked keys become exactly 0.
#     ACT is the bottleneck engine (~1us per [128,1024] exp call,
#     output-dtype independent, HW-measured).
#   - mm2: oaugT[d', q] += Vaug_tile.T @ attnT accumulated in PSUM,
#     Vaug = [V | 1] so row 64 carries the softmax denominator.
#   - Tail: PSUM escape (DVE), PE-transpose back to [q, d'], multiply by
#     1/denominator, DMA out.
#
# Engine-overlap schedule (the big win over the naive ordering):
#   - software pipeline with LAG=2: mm2 of k-tile pair p is emitted
#     after mm1+exp of pairs p+1, p+2, so the PE never sits behind the
#     exp it depends on and ACT runs back-to-back across slot
#     boundaries; each slot's tail is emitted as soon as its last mm2
#     is out (PSUM WAR deps on the shared pacc ring order it before the
#     next slot's first mm2).
#   - slot loads are prefetched one slot ahead (emitted at the previous
#     slot's pair 1), with the first k-tile pair's K chunk DMA'd
#     separately so pair-0 compute starts as soon as it lands.

import numpy as np
import ml_dtypes
from contextlib import ExitStack

import concourse.bass as bass
import concourse.bacc as bacc
import concourse.mybir as mybir
import concourse.tile as tile
from concourse.masks import make_identity
from concourse.bass_utils import run_bass_kernel_spmd

P = 128
S = 2048
D = 64
NT = S // P
NCORES = 8
NSLOTS = 4
MASK_NEG = -1.0e6
SCALE = 0.125
F32 = mybir.dt.float32
F32R = mybir.dt.float32r
FP16 = mybir.dt.float16

MM1_FP16 = True

LAST_RESULT = None
_PROGRAM_CACHE = {}


def _build_program(kcs, repeat=1, loop=1):
    nc = bacc.Bacc("TRN2", target_bir_lowering=False, debug=False)

    qk_dt = FP16 if MM1_FP16 else F32R
    qt_d = nc.dram_tensor("qt", [NSLOTS, D, S], qk_dt, kind="ExternalInput")
    kt_d = nc.dram_tensor("kt", [NSLOTS, D, S], qk_dt, kind="ExternalInput")
    v_d = nc.dram_tensor("v", [NSLOTS, P, NT, D], FP16, kind="ExternalInput")
    m_d = nc.dram_tensor("mask", [NSLOTS, P, NT], F32, kind="ExternalInput")
    o_d = nc.dram_tensor("out", [NSLOTS, P, NT, D], F32, kind="ExternalOutput")

    with ExitStack() as ctx:
        tc = ctx.enter_context(tile.TileContext(nc))
        consts = ctx.enter_context(tc.tile_pool(name="consts", bufs=1))
        tp = ctx.enter_context(tc.tile_pool(name="tp", bufs=2))
        vp = ctx.enter_context(tc.tile_pool(name="vp", bufs=2))
        atp = ctx.enter_context(tc.tile_pool(name="atp", bufs=6))
        op_ = ctx.enter_context(tc.tile_pool(name="op_", bufs=2))
        sm = ctx.enter_context(tc.tile_pool(name="sm", bufs=2))
        pmm = ctx.enter_context(tc.tile_pool(name="pmm", bufs=2, space="PSUM"))
        pacc = ctx.enter_context(tc.tile_pool(name="pacc", bufs=1, space="PSUM"))

        ident = consts.tile([P, P], F32)
        make_identity(nc, ident)

        # Per-slot live state: set by emit_load / first emit_mm2.
        st = {}

        def emit_load(k):
            s = k % NSLOTS
            kc = kcs[s]
            npr = (kc + 1) // 2
            nfull = kc // 2
            qt = tp.tile([D, S], qk_dt, tag="qt", name=f"qt{k}")
            nc.sync.dma_start(out=qt, in_=qt_d[s])
            mask_sb = sm.tile([P, NT], F32, tag="mask", name=f"mask{k}")
            nc.sync.dma_start(out=mask_sb, in_=m_d[s])
            # ktp/vaug split per pair so pair-0 compute starts as soon as
            # its chunk lands (kills the cold-start ACT gap).
            ktp = tp.tile([D, NT, P], qk_dt, tag="ktp", name=f"ktp{k}")
            kt_tiles = kt_d[s].rearrange("d (t p) -> d t p", p=P)
            vaug = vp.tile([P, NT, D + 1], FP16, tag="vaug", name=f"vaug{k}")
            nc.vector.memset(vaug[:, 0:kc, D : D + 1], 1.0)
            v_tiles = v_d[s]
            # first pair's K tiles land first so mm1 starts early
            nc.sync.dma_start(
                out=ktp[:, 0 : min(2, kc), :], in_=kt_tiles[:, 0 : min(2, kc), :]
            )
            if kc > 2:
                nc.sync.dma_start(
                    out=ktp[:, 2:kc, :], in_=kt_tiles[:, 2:kc, :]
                )
            nc.sync.dma_start(
                out=vaug[:, 0:kc, 0:D], in_=v_tiles[:, 0:kc, :]
            )
            st[k] = dict(
                kc=kc, npr=npr, qt=qt, ktp=ktp, vaug=vaug, mask=mask_sb,
                oaug=None, attn={},
            )

        def emit_mm1_exp(k, pr):
            z = st[k]
            kc, qt, ktp, mask_sb = z["kc"], z["qt"], z["ktp"], z["mask"]
            ka, kb = 2 * pr, 2 * pr + 1
            has_b = kb < kc
            attnA = atp.tile([P, S], FP16, tag="attnT", name=f"at{k}_{ka}")
            attnB = (
                atp.tile([P, S], FP16, tag="attnT", name=f"at{k}_{kb}")
                if has_b
                else None
            )
            z["attn"][ka] = attnA
            if has_b:
                z["attn"][kb] = attnB
            for h in range(2):
                psa = pmm.tile([P, 1024], F32, tag="pmm", name="psa")
                psb = (
                    pmm.tile([P, 1024], F32, tag="pmm", name="psb")
                    if has_b
                    else None
                )
                for j in range(2):
                    q_sl = slice(h * 1024 + j * 512, h * 1024 + (j + 1) * 512)
                    p_sl = slice(j * 512, (j + 1) * 512)
                    nc.tensor.matmul(
                        psa[:, p_sl], ktp[:, ka, :], qt[:, q_sl],
                        start=True, stop=True,
                    )
                    if has_b:
                        nc.tensor.matmul(
                            psb[:, p_sl], ktp[:, kb, :], qt[:, q_sl],
                            start=True, stop=True,
                        )
                h_sl = slice(h * 1024, (h + 1) * 1024)
                nc.scalar.activation(
                    out=attnA[:, h_sl], in_=psa,
                    func=mybir.ActivationFunctionType.Exp,
                    bias=mask_sb[:, ka : ka + 1], scale=SCALE,
                )
                if has_b:
                    nc.scalar.activation(
                        out=attnB[:, h_sl], in_=psb,
                        func=mybir.ActivationFunctionType.Exp,
                        bias=mask_sb[:, kb : kb + 1], scale=SCALE,
                    )

        def emit_mm2(k, pr):
            z = st[k]
            kc, vaug = z["kc"], z["vaug"]
            if z["oaug"] is None:
                z["oaug"] = pacc.tile(
                    [D + 1, S], F32, tag="oaug", name=f"oaug{k}"
                )
            oaug = z["oaug"]
            for kt_i in (2 * pr, 2 * pr + 1):
                attnT = z["attn"].pop(kt_i, None)
                if attnT is None:
                    continue
                for j in range(4):
                    nc.tensor.matmul(
                        oaug[:, j * 512 : (j + 1) * 512],
                        vaug[:, kt_i, :],
                        attnT[:, j * 512 : (j + 1) * 512],
                        start=(kt_i == 0),
                        stop=(kt_i == kc - 1),
                    )

        def emit_tail(k):
            s = k % NSLOTS
            z = st.pop(k)
            oaug = z["oaug"]
            oaug_sb = op_.tile([D + 1, S], F32, tag="oaugsb", name=f"ob{k}")
            nc.vector.tensor_copy(oaug_sb, oaug)
            out_sb = op_.tile([P, NT, D], F32, tag="outsb", name=f"os{k}")
            recip = sm.tile([P, NT], F32, tag="recip", name=f"rc{k}")
            # one padded tro tile (512B stride -> no transpose output
            # crosses a PSUM bank); single PE burst + one recip, killing
            # the 4-round PE<->DVE ping-pong in the tail chain.
            tro = pacc.tile([P, NT, P], F32, tag="oaug", name="tro")
            for qi in range(NT):
                nc.tensor.transpose(
                    tro[:, qi, 0 : D + 1],
                    oaug_sb[:, qi * P : (qi + 1) * P],
                    ident[0 : D + 1, 0 : D + 1],
                )
            nc.vector.reciprocal(recip, tro[:, :, D : D + 1])
            for qi in range(NT):
                nc.vector.tensor_scalar_mul(
                    out_sb[:, qi, :], tro[:, qi, 0:D], recip[:, qi : qi + 1]
                )
            nc.sync.dma_start(out=o_d[s], in_=out_sb)

        if loop > 1:
            ctx.enter_context(tc.For_i(0, loop))
        # Software pipeline across (rep, slot, pair) steps: mm2 trails
        # mm1+exp by LAG steps (keeps ACT fed across slot AND repeat
        # boundaries); each slot's tail is emitted as soon as its last
        # mm2 is out. Slot loads prefetch one slot ahead, wrapping into
        # the next repeat.
        LAG = 2
        steps = []
        for r in range(repeat):
            for s in range(NSLOTS):
                npr = (kcs[s] + 1) // 2
                for pr in range(npr):
                    steps.append((r * NSLOTS + s, pr, pr == npr - 1))
        pending = []  # (slot_key, pr, is_last) with mm2 not yet emitted

        def drain_one():
            k_, pr_, last_ = pending.pop(0)
            emit_mm2(k_, pr_)
            if last_:
                emit_tail(k_)

        nslots_total = repeat * NSLOTS
        emit_load(0)
        for k, pr, is_last in steps:
            npr_s = (kcs[k % NSLOTS] + 1) // 2
            if k + 1 < nslots_total and pr == min(1, npr_s - 1):
                emit_load(k + 1)
            emit_mm1_exp(k, pr)
            pending.append((k, pr, is_last))
            if len(pending) > LAG:
                drain_one()
        while pending:
            drain_one()

    nc.compile()
    return nc


def _plan(valid_lens):
    vl = np.asarray(valid_lens).astype(np.int64)
    order = np.argsort(-vl, kind="stable")
    assign = order.reshape(NSLOTS, NCORES)
    kcs = []
    for s_ in range(NSLOTS):
        m = int(vl[assign[s_]].max())
        kcs.append(max(1, -(-m // P)))
    return assign, kcs


def make_in_maps(queries, keys, values, vl, assign):
    key_ids = np.arange(S, dtype=np.int64)
    qk_np = np.float16 if MM1_FP16 else np.float32
    in_maps = []
    for c in range(NCORES):
        bidx = assign[:, c]
        mask = np.where(
            key_ids[None, :] < vl[bidx][:, None], 0.0, MASK_NEG
        ).astype(np.float32)
        mask = mask.reshape(NSLOTS, NT, P).transpose(0, 2, 1)
        in_maps.append(
            {
                "qt": np.ascontiguousarray(
                    queries[bidx].transpose(0, 2, 1).astype(qk_np)
                ),
                "kt": np.ascontiguousarray(
                    keys[bidx].transpose(0, 2, 1).astype(qk_np)
                ),
                "v": np.ascontiguousarray(
                    values[bidx]
                    .reshape(NSLOTS, NT, P, D)
                    .transpose(0, 2, 1, 3)
                    .astype(np.float16)
                ),
                "mask": np.ascontiguousarray(mask),
            }
        )
    return in_maps


def kernel(queries, keys, values, valid_lens):
    global LAST_RESULT
    queries = np.ascontiguousarray(np.asarray(queries), dtype=np.float32)
    keys = np.ascontiguousarray(np.asarray(keys), dtype=np.float32)
    values = np.ascontiguousarray(np.asarray(values), dtype=np.float32)
    vl = np.asarray(valid_lens).astype(np.int64)
    B = queries.shape[0]
    assert queries.shape == (B, S, D) and B == NCORES * NSLOTS

    assign, kcs = _plan(vl)
    key = tuple(kcs)
    nc = _PROGRAM_CACHE.get(key)
    if nc is None:
        nc = _PROGRAM_CACHE[key] = _build_program(kcs)
    in_maps = make_in_maps(queries, keys, values, vl, assign)

    import os
    try:
        LAST_RESULT = run_bass_kernel_spmd(
            nc, in_maps, core_ids=list(range(NCORES))
        )
    except ModuleNotFoundError:
        os.environ["BASS_NEVER_TRACE"] = "1"
        LAST_RESULT = run_bass_kernel_spmd(
            nc, in_maps, core_ids=list(range(NCORES))
        )

    out = np.empty((B, S, D), dtype=np.float32)
    for c in range(NCORES):
        o = LAST_RESULT.results[c]["out"]  # [NSLOTS, P, NT, D]
        for s_ in range(NSLOTS):
            out[assign[s_, c]] = (
                o[s_].transpose(1, 0, 2).reshape(S, D)
            )
    return out

